# revision 1
# baseline (speedup 1.0000x reference)
"""Trainium2 Bass kernel for nn_CrossScaleOrthogonalMambaBlock.

Contract: kernel(**inputs) takes the FULL unsharded inputs (numpy), returns the
FULL output [B, L, D] float32.  Internally shards batch-parallel across 8
NeuronCores (1 batch element per core), with all parameters replicated.

Self-contained: hardcodes all shapes; no sibling imports.
"""
import os
import sys
import numpy as np

sys.path.insert(0, "/opt/trn_rl_repo")

import concourse.bass as bass
from concourse import bacc
import concourse.mybir as mybir
import concourse.tile as tile
from concourse.bass_utils import run_bass_kernel_spmd
from concourse.masks import make_identity

# Problem dims
B, L, D, I, S, DTR, KC, PH = 8, 2048, 512, 256, 8, 32, 3, 128
NCORES = 8
NCH = L // 512          # 4 chunks of 512 along time
NBLK = 16               # channel blocks of 128 = (16 i's x 8 s's) per direction
THW = L // 2            # scan processes time in halves of 1024
F16 = mybir.dt.float16
F32 = mybir.dt.float32
AF = mybir.ActivationFunctionType
if os.environ.get("BASSK_SIMFUNCS"):
    class _AFSim:
        def __getattr__(self, n):
            if n == "Gelu": return mybir.ActivationFunctionType.Tanh
            if n == "Silu": return mybir.ActivationFunctionType.Sigmoid
            return getattr(mybir.ActivationFunctionType, n)
    AF = _AFSim()
OP = mybir.AluOpType

_CACHE: dict = {}


# --------------------------------------------------------------------------
# host-side parameter prep (pure layout: transpose / replicate / reorder)
# --------------------------------------------------------------------------

def _softplus(x):
    return np.logaddexp(0.0, x)


def _prep(inputs):
    f = lambda k: np.asarray(inputs[k], np.float32)
    x = f("x")
    position = f("position")
    ln_in_w, ln_in_b = f("ln_in_w"), f("ln_in_b")
    pos_w1, pos_b1 = f("pos_w1"), f("pos_b1")
    pos_w2, pos_b2 = f("pos_w2"), f("pos_b2")
    in_w = f("scan_in_w")
    cx_w, cz_w = f("scan_cx_w"), f("scan_cz_w")
    xp_w = f("scan_xp_w")
    dt_w, dt_b = f("scan_dt_w"), f("scan_dt_b")
    Alog, Dp = f("scan_Alog"), f("scan_Dp")
    out_w = f("scan_out_w")
    mix_w = f("mix_w")
    ln_out_w, ln_out_b = f("ln_out_w"), f("ln_out_b")

    ln1_id = bool(np.allclose(ln_in_w, 1.0) and np.allclose(ln_in_b, 0.0))
    ln2_id = bool(np.allclose(ln_out_w, 1.0) and np.allclose(ln_out_b, 0.0))
    sib_zero = bool(np.allclose(pos_b2, 0.0))

    w = {}
    w["pw1T"] = pos_w1.T.astype(np.float16)                      # [6, 128]
    w["pw2T"] = pos_w2.T.astype(np.float16)                      # [128, 512]
    w["gelub"] = pos_b1.reshape(PH, 1).astype(np.float32)        # [128, 1]
    w["sib"] = pos_b2.reshape(128, 4, order="F").astype(np.float32)  # [128, 4] (col=db)

    # in_w.T as [128, 2*4*512]: slice (d, kt) -> [128, 512]
    inwT = np.zeros((128, 2 * 4 * 512), np.float32)
    for d in range(2):
        t = in_w[d].T                                            # [din, dout]
        for kt in range(4):
            inwT[:, (d * 4 + kt) * 512:(d * 4 + kt + 1) * 512] = \
                t[kt * 128:(kt + 1) * 128, :]
    w["inwT"] = inwT.astype(np.float16)

    # conv taps as diagonal matrices [128, 2*4*3*128]; dir1 taps reversed
    convd = np.zeros((128, 2 * 4 * KC * 128), np.float32)
    for d in range(2):
        cw = np.concatenate([cx_w[d, :, 0, :], cz_w[d, :, 0, :]], 0)  # [512, 3]
        if d == 1:
            cw = cw[:, ::-1]
        for g in range(4):
            for t in range(KC):
                off = ((d * 4 + g) * KC + t) * 128
                dg = np.zeros((128, 128), np.float32)
                np.fill_diagonal(dg, cw[g * 128:(g + 1) * 128, t])
                convd[:, off:off + 128] = dg
    w["convd"] = convd.astype(np.float16)

    # xproj, reordered cols: 0:8 = b_raw, 32:40 = c_raw, 64:96 = dt_raw
    # layout [128, 2*2*96]: slice (d, kt) -> [128, 96]
    xpe = np.zeros((2, I, 96), np.float32)
    xpe[:, :, 0:8] = np.transpose(xp_w[:, DTR:DTR + S, :], (0, 2, 1))
    xpe[:, :, 32:40] = np.transpose(xp_w[:, DTR + S:DTR + 2 * S, :], (0, 2, 1))
    xpe[:, :, 64:96] = np.transpose(xp_w[:, 0:DTR, :], (0, 2, 1))
    xpwT = np.zeros((128, 2 * 2 * 96), np.float32)
    for d in range(2):
        for kt in range(2):
            xpwT[:, (d * 2 + kt) * 96:(d * 2 + kt + 1) * 96] = \
                xpe[d, kt * 128:(kt + 1) * 128, :]
    w["xpwT"] = xpwT.astype(np.float16)

    # dt_w.T [32, 2*256]: slice (d, mb) -> [32, 128]
    dtwT = np.zeros((96, 2 * 256), np.float32)
    for d in range(2):
        dtwT[64:96, d * 256:(d + 1) * 256] = dt_w[d].T
    w["dtwT"] = dtwT.astype(np.float16)
    # dt_b [128, 4]: col = d*2 + mb
    w["dtb"] = np.ascontiguousarray(
        dt_b.reshape(2, 2, 128).transpose(2, 0, 1).reshape(128, 4)).astype(np.float32)

    # expansion / reduction / replication masks
    p = np.arange(128)
    Estk = np.zeros((128, 8, 128), np.float32)   # E_j[r, p] = r == 16j + p//8
    Rstk = np.zeros((128, 8, 128), np.float32)   # R_j[p, m] = m == 16j + p//8
    for j in range(8):
        Estk[16 * j + p // 8, j, p] = 1.0
        Rstk[p, j, 16 * j + p // 8] = 1.0
    w["Estk"] = Estk.reshape(128, 8 * 128).astype(np.float16)
    w["Rstk"] = Rstk.reshape(128, 8 * 128).astype(np.float16)
    pat8 = np.zeros((128, 128), np.float32)
    pat8[0:8, :] = (np.arange(8)[:, None] == (p % 8)[None, :])
    pat8[32:40, :] = pat8[0:8, :]
    w["pat8"] = pat8.astype(np.float16)

    a = _softplus(Alog) + 1e-4                                   # [2, 256, 8]
    negaX = np.zeros((128, 32), np.float32)                      # col = d*16 + k
    for d in range(2):
        for k in range(16):
            negaX[:, d * 16 + k] = -a[d, 16 * k + p // 8, p % 8]
    w["negaX"] = negaX

    # out-proj extended rows [y(0:256); zb(256:512); xb*Dp(512:768)]
    # layout [128, 2*6*512]: slice (d, kt) -> [128, 512]
    owT = np.zeros((128, 2 * 6 * 512), np.float32)
    for d in range(2):
        ext = np.concatenate([out_w[d].T, out_w[d].T[0:256] * Dp[d][:, None]], 0)
        for kt in range(6):
            owT[:, (d * 6 + kt) * 512:(d * 6 + kt + 1) * 512] = \
                ext[kt * 128:(kt + 1) * 128, :]
    w["owT"] = owT.astype(np.float16)

    # mix_w.T [128, 12*512]: slice kt -> [128, 512]
    mixT = np.zeros((128, 12 * 512), np.float32)
    mt = mix_w.T                                                 # [1536, 512]
    for kt in range(12):
        mixT[:, kt * 512:(kt + 1) * 512] = mt[kt * 128:(kt + 1) * 128, :]
    w["mixT"] = mixT.astype(np.float16)

    w["ln1w"] = np.broadcast_to(ln_in_w, (128, D)).astype(np.float32).copy()
    w["ln1b"] = np.broadcast_to(ln_in_b, (128, D)).astype(np.float32).copy()
    w["ln2w"] = np.broadcast_to(ln_out_w, (128, D)).astype(np.float32).copy()
    w["ln2b"] = np.broadcast_to(ln_out_b, (128, D)).astype(np.float32).copy()

    flags = (ln1_id, ln2_id, sib_zero)
    return w, x, position, flags


# --------------------------------------------------------------------------
# device program
# --------------------------------------------------------------------------

WEIGHT_SHAPES = {
    "pw1T": ([6, PH], F16), "pw2T": ([PH, D], F16), "gelub": ([PH, 1], F32),
    "sib": ([128, 4], F32), "inwT": ([128, 2 * 4 * 512], F16),
    "convd": ([128, 2 * 4 * KC * 128], F16), "xpwT": ([128, 2 * 2 * 96], F16),
    "dtwT": ([96, 2 * 256], F16), "dtb": ([128, 4], F32),
    "Estk": ([128, 8 * 128], F16), "Rstk": ([128, 8 * 128], F16),
    "pat8": ([128, 128], F16), "negaX": ([128, 32], F32),
    "owT": ([128, 2 * 6 * 512], F16), "mixT": ([128, 12 * 512], F16),
    "ln1w": ([128, D], F32), "ln1b": ([128, D], F32),
    "ln2w": ([128, D], F32), "ln2b": ([128, D], F32),
}


def _build_nc(flags):
    ln1_id, ln2_id, sib_zero = flags
    nc = bacc.Bacc()

    def par(name, shape, dtype=F16, out=False):
        return nc.declare_dram_parameter(name, list(shape), dtype, isOutput=out).ap()

    H = {"flags": flags}
    H["x"] = par("x", [L, D], F32)
    H["posT"] = par("posT", [6, L], F16)
    skip = set()
    if ln1_id:
        skip |= {"ln1w", "ln1b"}
    if ln2_id:
        skip |= {"ln2w", "ln2b"}
    if sib_zero:
        skip |= {"sib"}
    H["skip"] = skip
    for nm, (shape, dt) in WEIGHT_SHAPES.items():
        if nm in skip:
            continue
        H[nm] = par(nm, shape, dt)
    H["out"] = par("out", [L, D], F32, out=True)

    dbg = os.environ.get("BASSK_DEBUG", "")
    H["dbg_outs"] = {}
    for spec in [s for s in dbg.split(",") if s]:
        name, *shape = spec.split(":")
        H["dbg_outs"][name] = par("dbg_" + name, [int(s) for s in shape],
                                  F16, out=True)

    with tile.TileContext(nc) as tc:
        _body(tc, H)
    nc.compile()
    return nc


def _body(tc, H):
    nc = tc.nc
    ln1_id, ln2_id, sib_zero = H["flags"]
    dbg_outs = H["dbg_outs"]

    def dump(name, src):
        if name in dbg_outs:
            nc.sync.dma_start(out=dbg_outs[name], in_=src)

    with (
        tc.tile_pool(name="wp", bufs=1) as wp,
        tc.tile_pool(name="ap", bufs=2) as ap,
        tc.tile_pool(name="sp", bufs=2) as sp,
        tc.tile_pool(name="mmps", bufs=2, space="PSUM") as mmps,
        tc.tile_pool(name="blkps", bufs=2, space="PSUM") as blkps,
        tc.tile_pool(name="yps", bufs=1, space="PSUM") as yps,
    ):
        # ---- persistent weights in SBUF ----
        wt = {}
        for nm, (shape, dt) in WEIGHT_SHAPES.items():
            if nm in H["skip"] or nm == "mixT":
                continue
            t = wp.tile(shape, dt, tag=nm)
            nc.sync.dma_start(out=t, in_=H[nm])
            wt[nm] = t

        ident32 = wp.tile([128, 128], F32, tag="ident32")
        make_identity(nc, ident32)
        eps_t = wp.tile([128, 1], F32, tag="eps")
        nc.vector.memset(eps_t, 1e-6)
        posT_t = wp.tile([6, L], F16, tag="posT")
        nc.sync.dma_start(out=posT_t, in_=H["posT"])

        # =================================================================
        # Stage A/B: LN1 of x (L-layout) -> normL fp16 [128, 512] x 16
        # =================================================================
        normL = []
        for m in range(16):
            xt = ap.tile([128, D], F32, tag="xin", bufs=2)
            nc.sync.dma_start(out=xt, in_=H["x"][m * 128:(m + 1) * 128, :])
            stats = ap.tile([128, 6], F32, tag="stats", bufs=3)
            nc.vector.bn_stats(out=stats, in_=xt)
            mv = ap.tile([128, 2], F32, tag="mv", bufs=3)
            nc.vector.bn_aggr(out=mv, in_=stats)
            nc.scalar.activation(out=mv[:, 1:2], in_=mv[:, 1:2], func=AF.Sqrt,
                                 bias=eps_t, scale=1.0)
            nc.vector.reciprocal(out=mv[:, 1:2], in_=mv[:, 1:2])
            nt = ap.tile([128, D], F32, tag="normL", bufs=6)
            if ln1_id:
                nc.vector.tensor_scalar(out=nt, in0=xt, scalar1=mv[:, 0:1],
                                        scalar2=mv[:, 1:2],
                                        op0=OP.subtract, op1=OP.mult)
            else:
                tmp = ap.tile([128, D], F32, tag="lntmp", bufs=2)
                nc.vector.tensor_scalar(out=tmp, in0=xt, scalar1=mv[:, 0:1],
                                        scalar2=mv[:, 1:2],
                                        op0=OP.subtract, op1=OP.mult)
                nc.vector.tensor_tensor(out=tmp, in0=tmp, in1=wt["ln1w"],
                                        op=OP.mult)
                nc.vector.tensor_tensor(out=nt, in0=tmp, in1=wt["ln1b"],
                                        op=OP.add)
            normL.append(nt)

        # =================================================================
        # Stage C: pos MLP layer 1: h1T = gelu(pw1T.T @ posT + b1)
        # =================================================================
        h1T = ap.tile([128, L], F16, tag="prow", bufs=2)
        for c in range(NCH):
            ps = mmps.tile([128, 512], F32, tag="mm")
            nc.tensor.matmul(ps, wt["pw1T"], posT_t[:, c * 512:(c + 1) * 512],
                             start=True, stop=True)
            nc.scalar.activation(out=h1T[:, c * 512:(c + 1) * 512], in_=ps,
                                 func=AF.Gelu, bias=wt["gelub"], scale=1.0)

        # =================================================================
        # Stage D: siT[db] = blockwise-transpose(normL) + pw2T.T @ h1T (+sib)
        # =================================================================
        siT = [ap.tile([128, L], F16, tag=f"siT{db}", bufs=1, name=f"siT{db}")
               for db in range(4)]
        for c in range(NCH):
            for db in range(4):
                st = siT[db]
                ps = mmps.tile([128, 512], F32, tag="mm")
                nc.tensor.matmul(ps, wt["pw2T"][:, db * 128:(db + 1) * 128],
                                 h1T[:, c * 512:(c + 1) * 512],
                                 start=True, stop=False)
                for q in range(4):
                    nc.tensor.matmul(ps[:, q * 128:(q + 1) * 128],
                                     normL[c * 4 + q][:, db * 128:(db + 1) * 128],
                                     ident32, is_transpose=True,
                                     start=False, stop=(q == 3))
                dst = st[:, c * 512:(c + 1) * 512]
                if sib_zero:
                    nc.scalar.activation(out=dst, in_=ps, func=AF.Copy)
                else:
                    nc.scalar.activation(out=dst, in_=ps, func=AF.Identity,
                                         bias=wt["sib"][:, db:db + 1], scale=1.0)
        dump("siT", siT[0])

        # =================================================================
        # Stage E: per-direction scan pipeline
        # =================================================================
        fwdT, bwdT = [], []
        for d in range(2):
            outT = fwdT if d == 0 else bwdT

            # ---- in-proj into padded conv-input tiles ----
            xzpad = []
            for g in range(4):
                xt = ap.tile([128, L + 2], F16, tag="xzpad", bufs=4)
                xzpad.append(xt)
                nc.gpsimd.memset(xt[:, 0:1], 0.0)
                nc.gpsimd.memset(xt[:, L + 1:L + 2], 0.0)
                for c in range(NCH):
                    ps = mmps.tile([128, 512], F32, tag="mm")
                    for kt in range(4):
                        nc.tensor.matmul(
                            ps,
                            wt["inwT"][:, (d * 4 + kt) * 512 + g * 128:
                                       (d * 4 + kt) * 512 + (g + 1) * 128],
                            siT[kt][:, c * 512:(c + 1) * 512],
                            start=(kt == 0), stop=(kt == 3))
                    nc.scalar.activation(
                        out=xt[:, 1 + c * 512:1 + (c + 1) * 512],
                        in_=ps, func=AF.Copy)

            # ---- depthwise conv (diag matmuls) + SiLU ----
            xbT, zbT = [], []
            for g in range(4):
                ot = ap.tile([128, L], F16, tag="xbzb", bufs=5)
                (xbT if g < 2 else zbT).append(ot)
                for c in range(NCH):
                    ps = mmps.tile([128, 512], F32, tag="mm")
                    for t in range(KC):
                        off = ((d * 4 + g) * KC + t) * 128
                        nc.tensor.matmul(
                            ps, wt["convd"][:, off:off + 128],
                            xzpad[g][:, c * 512 + t:c * 512 + t + 512],
                            start=(t == 0), stop=(t == KC - 1))
                    nc.scalar.activation(out=ot[:, c * 512:(c + 1) * 512],
                                         in_=ps, func=AF.Silu)
            if d == 0:
                dump("xbT", xbT[0])

            # ---- xproj -> prow: bt(0:8), ct(32:40), dt_raw(64:96) ----
            prow = ap.tile([128, L], F16, tag="prow", bufs=2, name=f"prow{d}")
            for c in range(NCH):
                ps = mmps.tile([128, 512], F32, tag="mm")
                for kt in range(2):
                    nc.tensor.matmul(
                        ps[0:96, :],
                        wt["xpwT"][:, (d * 2 + kt) * 96:(d * 2 + kt + 1) * 96],
                        xbT[kt][:, c * 512:(c + 1) * 512],
                        start=(kt == 0), stop=(kt == 1))
                sl = slice(c * 512, (c + 1) * 512)
                nc.scalar.activation(out=prow[0:8, sl], in_=ps[0:8, :],
                                     func=AF.Tanh)
                nc.scalar.activation(out=prow[32:40, sl], in_=ps[32:40, :],
                                     func=AF.Tanh)
                nc.scalar.activation(out=prow[64:96, sl], in_=ps[64:96, :],
                                     func=AF.Copy)

            # ---- btR/ctR: replicate bt/ct across partitions (s = p%8) ----
            btR = ap.tile([128, L], F16, tag="btR", bufs=1)
            ctR = ap.tile([128, L], F16, tag="ctR", bufs=1)
            for c in range(NCH):
                sl = slice(c * 512, (c + 1) * 512)
                ps = mmps.tile([128, 512], F32, tag="mm")
                nc.tensor.matmul(ps, wt["pat8"][0:8, :], prow[0:8, sl],
                                 start=True, stop=True)
                nc.vector.tensor_copy(btR[:, sl], ps)
                ps2 = mmps.tile([128, 512], F32, tag="mm")
                nc.tensor.matmul(ps2, wt["pat8"][32:40, :], prow[32:40, sl],
                                 start=True, stop=True)
                nc.vector.tensor_copy(ctR[:, sl], ps2)

            # ---- delta = softplus(dtwT.T @ dt_raw + dtb) [2 x 128, L] ----
            deltaT = []
            for mb in range(2):
                dt_t = ap.tile([128, L], F16, tag="deltaT", bufs=2)
                deltaT.append(dt_t)
                for c in range(NCH):
                    ps = mmps.tile([128, 512], F32, tag="mm")
                    nc.tensor.matmul(
                        ps, wt["dtwT"][64:96, d * 256 + mb * 128:
                                       d * 256 + (mb + 1) * 128],
                        prow[64:96, c * 512:(c + 1) * 512],
                        start=True, stop=True)
                    # softplus(x + b) = ln(1 + exp(x + b)), exact
                    nc.scalar.activation(
                        out=ps, in_=ps, func=AF.Exp,
                        bias=wt["dtb"][:, 2 * d + mb:2 * d + mb + 1], scale=1.0)
                    nc.scalar.activation(
                        out=dt_t[:, c * 512:(c + 1) * 512], in_=ps,
                        func=AF.Ln, bias=1.0, scale=1.0)
            if d == 0:
                dump("deltaT", deltaT[0])

            # ---- scan core: 16 channel blocks x 2 time halves ----
            yT = [ap.tile([128, L], F16, tag="yT", bufs=2, name=f"yT{d}_{ih}")
                  for ih in range(2)]
            carry = ap.tile([128, 16], F16, tag="carry", bufs=2)
            th_order = (0, 1) if d == 0 else (1, 0)
            ypt = [None, None]
            for thi, th in enumerate(th_order):
                t0 = th * THW
                for k in range(NBLK):
                    ih, j = k // 8, k % 8
                    esl = slice(j * 128, (j + 1) * 128)
                    dx = blkps.tile([128, THW], F32, tag="blk")
                    for h2 in range(2):
                        nc.tensor.matmul(
                            dx[:, h2 * 512:(h2 + 1) * 512], wt["Estk"][:, esl],
                            deltaT[ih][:, t0 + h2 * 512:t0 + (h2 + 1) * 512],
                            start=True, stop=True)
                    d_t = sp.tile([128, THW], F16, tag="d", bufs=2)
                    nc.scalar.activation(
                        out=d_t, in_=dx, func=AF.Exp,
                        scale=wt["negaX"][:, d * 16 + k:d * 16 + k + 1])
                    xx = blkps.tile([128, THW], F32, tag="blk")
                    for h2 in range(2):
                        nc.tensor.matmul(
                            xx[:, h2 * 512:(h2 + 1) * 512], wt["Estk"][:, esl],
                            xbT[ih][:, t0 + h2 * 512:t0 + (h2 + 1) * 512],
                            start=True, stop=True)
                    xbx = sp.tile([128, THW], F16, tag="xbx", bufs=2)
                    nc.vector.tensor_copy(xbx, xx)
                    w_t = sp.tile([128, THW], F16, tag="w", bufs=2)
                    nc.vector.tensor_tensor(out=w_t, in0=btR[:, t0:t0 + THW],
                                            in1=xbx, op=OP.mult)
                    e_t = sp.tile([128, THW], F16, tag="e", bufs=2)
                    nc.vector.tensor_scalar(out=e_t, in0=d_t, scalar1=-1.0,
                                            scalar2=1.0, op0=OP.mult, op1=OP.add)
                    u_t = e_t
                    nc.gpsimd.tensor_tensor(out=u_t, in0=e_t, in1=w_t, op=OP.mult)
                    h_t = sp.tile([128, THW], F16, tag="h", bufs=2)
                    init = 0.0 if thi == 0 else carry[:, k:k + 1]
                    if d == 0:
                        nc.vector.tensor_tensor_scan(
                            out=h_t, data0=d_t, data1=u_t, initial=init,
                            op0=OP.mult, op1=OP.add)
                        nc.vector.tensor_copy(carry[:, k:k + 1],
                                              h_t[:, THW - 1:THW])
                    else:
                        nc.vector.tensor_tensor_scan(
                            out=h_t[:, ::-1], data0=d_t[:, ::-1],
                            data1=u_t[:, ::-1], initial=init,
                            op0=OP.mult, op1=OP.add)
                        nc.vector.tensor_copy(carry[:, k:k + 1], h_t[:, 0:1])
                    z_t = h_t
                    nc.gpsimd.tensor_tensor(out=z_t, in0=h_t,
                                            in1=ctR[:, t0:t0 + THW], op=OP.mult)
                    if j == 0:
                        ypt[ih] = yps.tile([128, THW], F32, tag="y",
                                           name=f"yp{d}_{th}_{ih}")
                    for h2 in range(2):
                        nc.tensor.matmul(
                            ypt[ih][:, h2 * 512:(h2 + 1) * 512],
                            wt["Rstk"][:, esl], z_t[:, h2 * 512:(h2 + 1) * 512],
                            start=(j == 0), stop=(j == 7))
                    if j == 7:
                        nc.scalar.activation(out=yT[ih][:, t0:t0 + THW],
                                             in_=ypt[ih], func=AF.Copy)
            if d == 0:
                dump("yT", yT[0])

            # ---- out-proj ----
            rhs_tiles = [yT[0], yT[1], zbT[0], zbT[1], xbT[0], xbT[1]]
            for db in range(4):
                ot = ap.tile([128, L], F16, tag=f"proj{d}_{db}", bufs=1)
                outT.append(ot)
                for c in range(NCH):
                    ps = mmps.tile([128, 512], F32, tag="mm")
                    for kt in range(6):
                        nc.tensor.matmul(
                            ps,
                            wt["owT"][:, (d * 6 + kt) * 512 + db * 128:
                                      (d * 6 + kt) * 512 + (db + 1) * 128],
                            rhs_tiles[kt][:, c * 512:(c + 1) * 512],
                            start=(kt == 0), stop=(kt == 5))
                    nc.vector.tensor_copy(ot[:, c * 512:(c + 1) * 512], ps)
        dump("fwdT", fwdT[0])
        dump("bwdT", bwdT[0])

        # =================================================================
        # Stage F/G: mix matmul (L-layout out) + LN2 + store
        # =================================================================
        mixT_t = wp.tile([128, 12 * 512], F16, tag="mixT")
        nc.sync.dma_start(out=mixT_t, in_=H["mixT"])
        lhs_tiles = fwdT + bwdT + siT
        for m in range(16):
            ps = mmps.tile([128, 512], F32, tag="mm")
            for kt in range(12):
                nc.tensor.matmul(ps, lhs_tiles[kt][:, m * 128:(m + 1) * 128],
                                 mixT_t[:, kt * 512:(kt + 1) * 512],
                                 start=(kt == 0), stop=(kt == 11))
            stats = ap.tile([128, 6], F32, tag="stats2", bufs=3)
            nc.vector.bn_stats(out=stats, in_=ps)
            mv = ap.tile([128, 2], F32, tag="mv2", bufs=3)
            nc.vector.bn_aggr(out=mv, in_=stats)
            nc.scalar.activation(out=mv[:, 1:2], in_=mv[:, 1:2], func=AF.Sqrt,
                                 bias=eps_t, scale=1.0)
            nc.vector.reciprocal(out=mv[:, 1:2], in_=mv[:, 1:2])
            ot = ap.tile([128, D], F32, tag="outL", bufs=2)
            nc.vector.tensor_scalar(out=ot, in0=ps, scalar1=mv[:, 0:1],
                                    scalar2=mv[:, 1:2],
                                    op0=OP.subtract, op1=OP.mult)
            if not ln2_id:
                nc.vector.tensor_tensor(out=ot, in0=ot, in1=wt["ln2w"],
                                        op=OP.mult)
                nc.vector.tensor_tensor(out=ot, in0=ot, in1=wt["ln2b"],
                                        op=OP.add)
            nc.sync.dma_start(out=H["out"][m * 128:(m + 1) * 128, :], in_=ot)


# --------------------------------------------------------------------------
# entry point
# --------------------------------------------------------------------------

def _get_nc(flags):
    key = ("nc", flags, os.environ.get("BASSK_DEBUG", ""))
    if key not in _CACHE:
        _CACHE[key] = _build_nc(flags)
    return _CACHE[key]


def make_in_maps(inputs):
    w, x, position, flags = _prep(inputs)
    shared = {k: v for k, v in w.items() if isinstance(v, np.ndarray)}
    ln1_id, ln2_id, sib_zero = flags
    if ln1_id:
        shared.pop("ln1w"), shared.pop("ln1b")
    if ln2_id:
        shared.pop("ln2w"), shared.pop("ln2b")
    if sib_zero:
        shared.pop("sib")
    in_maps = []
    for b in range(NCORES):
        m = dict(shared)
        m["x"] = np.ascontiguousarray(x[b])
        m["posT"] = np.ascontiguousarray(position[b].T).astype(np.float16)
        in_maps.append(m)
    return in_maps, flags


def kernel(**inputs):
    in_maps, flags = make_in_maps(inputs)
    nc = _get_nc(flags)
    res = run_bass_kernel_spmd(nc, in_maps, list(range(NCORES)))
    out = np.stack([np.asarray(res.results[b]["out"]) for b in range(NCORES)])
    return out.astype(np.float32)


if __name__ == "__main__":
    import time
    t0 = time.time()
    nc = _get_nc((True, True, True))
    n_inst = len(nc.m.functions[0].instructions)
    print(f"build ok in {time.time() - t0:.1f}s")



# revision 17
# speedup vs baseline: 4457.3033x; 4457.3033x over previous
"""Trainium2 Bass kernel for nn_CrossScaleOrthogonalMambaBlock.

Contract: kernel(**inputs) takes the FULL unsharded inputs (numpy), returns the
FULL output [B, L, D] float32.  Internally shards batch-parallel across 8
NeuronCores (1 batch element per core), with all parameters replicated.

Self-contained: hardcodes all shapes; no sibling imports.
"""
import os
import sys
import numpy as np

sys.path.insert(0, "/opt/trn_rl_repo")

import concourse.bass as bass
from concourse import bacc
import concourse.mybir as mybir
import concourse.tile as tile
from concourse.bass_utils import run_bass_kernel_spmd
from concourse.masks import make_identity

# Problem dims
B, L, D, I, S, DTR, KC, PH = 8, 2048, 512, 256, 8, 32, 3, 128
NCORES = 8
NCH = L // 512          # 4 chunks of 512 along time
NBLK = 16               # channel blocks of 128 = (16 i's x 8 s's) per direction
THW = L // 2            # scan processes time in halves of 1024
F16 = mybir.dt.float16
F32 = mybir.dt.float32
AF = mybir.ActivationFunctionType
if os.environ.get("BASSK_SIMFUNCS"):
    class _AFSim:
        def __getattr__(self, n):
            if n == "Gelu": return mybir.ActivationFunctionType.Tanh
            if n == "Silu": return mybir.ActivationFunctionType.Sigmoid
            return getattr(mybir.ActivationFunctionType, n)
    AF = _AFSim()
OP = mybir.AluOpType

_CACHE: dict = {}


# --------------------------------------------------------------------------
# host-side parameter prep (pure layout: transpose / replicate / reorder)
# --------------------------------------------------------------------------

def _softplus(x):
    return np.logaddexp(0.0, x)


def _prep(inputs):
    f = lambda k: np.asarray(inputs[k], np.float32)
    x = f("x")
    position = f("position")
    ln_in_w, ln_in_b = f("ln_in_w"), f("ln_in_b")
    pos_w1, pos_b1 = f("pos_w1"), f("pos_b1")
    pos_w2, pos_b2 = f("pos_w2"), f("pos_b2")
    in_w = f("scan_in_w")
    cx_w, cz_w = f("scan_cx_w"), f("scan_cz_w")
    xp_w = f("scan_xp_w")
    dt_w, dt_b = f("scan_dt_w"), f("scan_dt_b")
    Alog, Dp = f("scan_Alog"), f("scan_Dp")
    out_w = f("scan_out_w")
    mix_w = f("mix_w")
    ln_out_w, ln_out_b = f("ln_out_w"), f("ln_out_b")

    ln1_id = bool(np.allclose(ln_in_w, 1.0) and np.allclose(ln_in_b, 0.0))
    ln2_id = bool(np.allclose(ln_out_w, 1.0) and np.allclose(ln_out_b, 0.0))
    sib_zero = bool(np.allclose(pos_b2, 0.0))

    w = {}
    w["pw1T"] = pos_w1.T.astype(np.float16)                      # [6, 128]
    w["pw2T"] = pos_w2.T.astype(np.float16)                      # [128, 512]
    w["gelub"] = pos_b1.reshape(PH, 1).astype(np.float32)        # [128, 1]
    w["sib"] = pos_b2.reshape(128, 4, order="F").astype(np.float32)  # [128, 4] (col=db)

    # in_w.T as [128, 2*4*512]: slice (d, kt) -> [128, 512]
    inwT = np.zeros((128, 2 * 4 * 512), np.float32)
    for d in range(2):
        t = in_w[d].T                                            # [din, dout]
        for kt in range(4):
            inwT[:, (d * 4 + kt) * 512:(d * 4 + kt + 1) * 512] = \
                t[kt * 128:(kt + 1) * 128, :]
    w["inwT"] = inwT.astype(np.float16)

    # conv taps as diagonal matrices [128, 2*4*3*128]; dir1 taps reversed
    convd = np.zeros((128, 2 * 4 * KC * 128), np.float32)
    for d in range(2):
        cw = np.concatenate([cx_w[d, :, 0, :], cz_w[d, :, 0, :]], 0)  # [512, 3]
        if d == 1:
            cw = cw[:, ::-1]
        for g in range(4):
            for t in range(KC):
                off = ((d * 4 + g) * KC + t) * 128
                dg = np.zeros((128, 128), np.float32)
                np.fill_diagonal(dg, cw[g * 128:(g + 1) * 128, t])
                convd[:, off:off + 128] = dg
    w["convd"] = convd.astype(np.float16)

    # xproj, reordered cols: 0:8 = b_raw, 32:40 = c_raw, 64:96 = dt_raw
    # layout [128, 2*2*96]: slice (d, kt) -> [128, 96]
    xpe = np.zeros((2, I, 96), np.float32)
    xpe[:, :, 0:8] = np.transpose(xp_w[:, DTR:DTR + S, :], (0, 2, 1))
    xpe[:, :, 32:40] = np.transpose(xp_w[:, DTR + S:DTR + 2 * S, :], (0, 2, 1))
    xpe[:, :, 64:96] = np.transpose(xp_w[:, 0:DTR, :], (0, 2, 1))
    xpwT = np.zeros((128, 2 * 2 * 96), np.float32)
    for d in range(2):
        for kt in range(2):
            xpwT[:, (d * 2 + kt) * 96:(d * 2 + kt + 1) * 96] = \
                xpe[d, kt * 128:(kt + 1) * 128, :]
    w["xpwT"] = xpwT.astype(np.float16)

    # dt_w.T [32, 2*256]: slice (d, mb) -> [32, 128]
    dtwT = np.zeros((96, 2 * 256), np.float32)
    for d in range(2):
        dtwT[64:96, d * 256:(d + 1) * 256] = dt_w[d].T
    w["dtwT"] = dtwT.astype(np.float16)
    # dt_b [128, 4]: col = d*2 + mb
    w["dtb"] = np.ascontiguousarray(
        dt_b.reshape(2, 2, 128).transpose(2, 0, 1).reshape(128, 4)).astype(np.float32)

    # expansion / reduction / replication masks
    p = np.arange(128)
    Estk = np.zeros((128, 8, 128), np.float32)   # E_j[r, p] = r == 16j + p//8
    Rstk = np.zeros((128, 8, 128), np.float32)   # R_j[p, m] = m == 16j + p//8
    for j in range(8):
        Estk[16 * j + p // 8, j, p] = 1.0
        Rstk[p, j, 16 * j + p // 8] = 1.0
    w["Estk"] = Estk.reshape(128, 8 * 128).astype(np.float16)
    w["Rstk"] = Rstk.reshape(128, 8 * 128).astype(np.float16)
    pat8 = np.zeros((128, 128), np.float32)
    # bt replication negated: btR = -bt, so u = (d-1)*(btR*xb) = (1-d)*bt*xb
    pat8[0:8, :] = -(np.arange(8)[:, None] == (p % 8)[None, :]).astype(np.float32)
    pat8[32:40, :] = (np.arange(8)[:, None] == (p % 8)[None, :])
    w["pat8"] = pat8.astype(np.float16)

    a = _softplus(Alog) + 1e-4                                   # [2, 256, 8]
    negaX = np.zeros((128, 32), np.float32)                      # col = d*16 + k
    for d in range(2):
        for k in range(16):
            negaX[:, d * 16 + k] = -a[d, 16 * k + p // 8, p % 8]
    w["negaX"] = negaX

    # out-proj extended rows [y(0:256); zb(256:512); xb*Dp(512:768)]
    # layout [128, 2*6*512]: slice (d, kt) -> [128, 512]
    owT = np.zeros((128, 2 * 6 * 512), np.float32)
    for d in range(2):
        ext = np.concatenate([out_w[d].T, out_w[d].T[0:256] * Dp[d][:, None]], 0)
        for kt in range(6):
            owT[:, (d * 6 + kt) * 512:(d * 6 + kt + 1) * 512] = \
                ext[kt * 128:(kt + 1) * 128, :]
    w["owT"] = owT.astype(np.float16)

    # mix_w.T [128, 12*512]: slice kt -> [128, 512]
    mixT = np.zeros((128, 12 * 512), np.float32)
    mt = mix_w.T                                                 # [1536, 512]
    for kt in range(12):
        mixT[:, kt * 512:(kt + 1) * 512] = mt[kt * 128:(kt + 1) * 128, :]
    w["mixT"] = mixT.astype(np.float16)

    w["ln1w"] = np.broadcast_to(ln_in_w, (128, D)).astype(np.float32).copy()
    w["ln1b"] = np.broadcast_to(ln_in_b, (128, D)).astype(np.float32).copy()
    w["ln2w"] = np.broadcast_to(ln_out_w, (128, D)).astype(np.float32).copy()
    w["ln2b"] = np.broadcast_to(ln_out_b, (128, D)).astype(np.float32).copy()

    flags = (ln1_id, ln2_id, sib_zero)
    return w, x, position, flags


# --------------------------------------------------------------------------
# device program
# --------------------------------------------------------------------------

WEIGHT_SHAPES = {
    "pw1T": ([6, PH], F16), "pw2T": ([PH, D], F16), "gelub": ([PH, 1], F32),
    "sib": ([128, 4], F32), "inwT": ([128, 2 * 4 * 512], F16),
    "convd": ([128, 2 * 4 * KC * 128], F16), "xpwT": ([128, 2 * 2 * 96], F16),
    "dtwT": ([96, 2 * 256], F16), "dtb": ([128, 4], F32),
    "Estk": ([128, 8 * 128], F16), "Rstk": ([128, 8 * 128], F16),
    "pat8": ([128, 128], F16), "negaX": ([128, 32], F32),
    "owT": ([128, 2 * 6 * 512], F16), "mixT": ([128, 12 * 512], F16),
    "ln1w": ([128, D], F32), "ln1b": ([128, D], F32),
    "ln2w": ([128, D], F32), "ln2b": ([128, D], F32),
}


def _build_nc(flags):
    ln1_id, ln2_id, sib_zero = flags
    nc = bacc.Bacc()

    def par(name, shape, dtype=F16, out=False):
        return nc.declare_dram_parameter(name, list(shape), dtype, isOutput=out).ap()

    H = {"flags": flags}
    H["x"] = par("x", [L, D], F32)
    H["posT"] = par("posT", [6, L], F16)
    skip = set()
    if ln1_id:
        skip |= {"ln1w", "ln1b"}
    if ln2_id:
        skip |= {"ln2w", "ln2b"}
    if sib_zero:
        skip |= {"sib"}
    H["skip"] = skip
    for nm, (shape, dt) in WEIGHT_SHAPES.items():
        if nm in skip:
            continue
        H[nm] = par(nm, shape, dt)
    H["out"] = par("out", [L, D], F32, out=True)

    dbg = os.environ.get("BASSK_DEBUG", "")
    H["dbg_outs"] = {}
    for spec in [s for s in dbg.split(",") if s]:
        name, *shape = spec.split(":")
        H["dbg_outs"][name] = par("dbg_" + name, [int(s) for s in shape],
                                  F16, out=True)

    with tile.TileContext(nc) as tc:
        _body(tc, H)
    nc.compile()
    return nc


def _body(tc, H):
    nc = tc.nc
    ln1_id, ln2_id, sib_zero = H["flags"]
    dbg_outs = H["dbg_outs"]

    def dump(name, src):
        if name in dbg_outs:
            nc.sync.dma_start(out=dbg_outs[name], in_=src)

    with (
        tc.tile_pool(name="wp", bufs=1) as wp,
        tc.tile_pool(name="ap", bufs=2) as ap,
        tc.tile_pool(name="sp", bufs=2) as sp,
        tc.tile_pool(name="mmps", bufs=2, space="PSUM") as mmps,
        tc.tile_pool(name="blkps", bufs=2, space="PSUM") as blkps,
        tc.tile_pool(name="yps", bufs=1, space="PSUM") as yps,
    ):
        # ---- persistent weights in SBUF ----
        wt = {}
        for nm, (shape, dt) in WEIGHT_SHAPES.items():
            if nm in H["skip"] or nm == "mixT":
                continue
            t = wp.tile(shape, dt, tag=nm)
            nc.sync.dma_start(out=t, in_=H[nm])
            wt[nm] = t

        ident32 = wp.tile([128, 128], F32, tag="ident32")
        make_identity(nc, ident32)
        eps_t = wp.tile([128, 1], F32, tag="eps")
        nc.vector.memset(eps_t, 1e-6)
        posT_t = wp.tile([6, L], F16, tag="posT")
        nc.sync.dma_start(out=posT_t, in_=H["posT"])

        # =================================================================
        # Stage A/B: LN1 of x (L-layout) -> normL fp16 [128, 512] x 16
        # =================================================================
        normL = []
        for m in range(16):
            xt = ap.tile([128, D], F32, tag="xin", bufs=2)
            nc.sync.dma_start(out=xt, in_=H["x"][m * 128:(m + 1) * 128, :])
            stats = ap.tile([128, 6], F32, tag="stats", bufs=3)
            nc.vector.bn_stats(out=stats, in_=xt)
            mv = ap.tile([128, 2], F32, tag="mv", bufs=3)
            nc.vector.bn_aggr(out=mv, in_=stats)
            nc.scalar.activation(out=mv[:, 1:2], in_=mv[:, 1:2], func=AF.Sqrt,
                                 bias=eps_t, scale=1.0)
            nc.vector.reciprocal(out=mv[:, 1:2], in_=mv[:, 1:2])
            nt = ap.tile([128, D], F32, tag="normL", bufs=6)
            if ln1_id:
                nc.vector.tensor_scalar(out=nt, in0=xt, scalar1=mv[:, 0:1],
                                        scalar2=mv[:, 1:2],
                                        op0=OP.subtract, op1=OP.mult)
            else:
                tmp = ap.tile([128, D], F32, tag="lntmp", bufs=2)
                nc.vector.tensor_scalar(out=tmp, in0=xt, scalar1=mv[:, 0:1],
                                        scalar2=mv[:, 1:2],
                                        op0=OP.subtract, op1=OP.mult)
                nc.vector.tensor_tensor(out=tmp, in0=tmp, in1=wt["ln1w"],
                                        op=OP.mult)
                nc.vector.tensor_tensor(out=nt, in0=tmp, in1=wt["ln1b"],
                                        op=OP.add)
            normL.append(nt)

        # =================================================================
        # Stage C: pos MLP layer 1: h1T = gelu(pw1T.T @ posT + b1)
        # =================================================================
        h1T = ap.tile([128, L], F16, tag="prow", bufs=2)
        for c in range(NCH):
            ps = mmps.tile([128, 512], F32, tag="mm")
            nc.tensor.matmul(ps, wt["pw1T"], posT_t[:, c * 512:(c + 1) * 512],
                             start=True, stop=True)
            nc.scalar.activation(out=h1T[:, c * 512:(c + 1) * 512], in_=ps,
                                 func=AF.Gelu, bias=wt["gelub"], scale=1.0)

        # =================================================================
        # Stage D: siT[db] = blockwise-transpose(normL) + pw2T.T @ h1T (+sib)
        # =================================================================
        siT = [ap.tile([128, L], F16, tag=f"siT{db}", bufs=1, name=f"siT{db}")
               for db in range(4)]
        for c in range(NCH):
            for db in range(4):
                st = siT[db]
                ps = mmps.tile([128, 512], F32, tag="mm")
                nc.tensor.matmul(ps, wt["pw2T"][:, db * 128:(db + 1) * 128],
                                 h1T[:, c * 512:(c + 1) * 512],
                                 start=True, stop=False)
                for q in range(4):
                    nc.tensor.matmul(ps[:, q * 128:(q + 1) * 128],
                                     normL[c * 4 + q][:, db * 128:(db + 1) * 128],
                                     ident32, is_transpose=True,
                                     start=False, stop=(q == 3))
                dst = st[:, c * 512:(c + 1) * 512]
                if sib_zero:
                    nc.scalar.activation(out=dst, in_=ps, func=AF.Copy)
                else:
                    nc.scalar.activation(out=dst, in_=ps, func=AF.Identity,
                                         bias=wt["sib"][:, db:db + 1], scale=1.0)
        dump("siT", siT[0])

        # =================================================================
        # Stage E: per-direction scan pipeline
        # =================================================================
        fwdT, bwdT = [], []
        for d in range(2):
            outT = fwdT if d == 0 else bwdT

            # ---- in-proj into padded conv-input tiles ----
            xzpad = []
            for g in range(4):
                xt = ap.tile([128, L + 2], F16, tag="xzpad", bufs=4)
                xzpad.append(xt)
                nc.gpsimd.memset(xt[:, 0:1], 0.0)
                nc.gpsimd.memset(xt[:, L + 1:L + 2], 0.0)
                for c in range(NCH):
                    ps = mmps.tile([128, 512], F32, tag="mm")
                    for kt in range(4):
                        nc.tensor.matmul(
                            ps,
                            wt["inwT"][:, (d * 4 + kt) * 512 + g * 128:
                                       (d * 4 + kt) * 512 + (g + 1) * 128],
                            siT[kt][:, c * 512:(c + 1) * 512],
                            start=(kt == 0), stop=(kt == 3))
                    nc.scalar.activation(
                        out=xt[:, 1 + c * 512:1 + (c + 1) * 512],
                        in_=ps, func=AF.Copy)

            # ---- depthwise conv (diag matmuls) + SiLU ----
            xbT, zbT = [], []
            for g in range(4):
                ot = ap.tile([128, L], F16, tag="xbzb", bufs=5)
                (xbT if g < 2 else zbT).append(ot)
                for c in range(NCH):
                    ps = mmps.tile([128, 512], F32, tag="mm")
                    for t in range(KC):
                        off = ((d * 4 + g) * KC + t) * 128
                        nc.tensor.matmul(
                            ps, wt["convd"][:, off:off + 128],
                            xzpad[g][:, c * 512 + t:c * 512 + t + 512],
                            start=(t == 0), stop=(t == KC - 1))
                    nc.scalar.activation(out=ot[:, c * 512:(c + 1) * 512],
                                         in_=ps, func=AF.Silu)
            if d == 0:
                dump("xbT", xbT[0])

            # ---- xproj -> prow: bt(0:8), ct(32:40), dt_raw(64:96) ----
            prow = ap.tile([128, L], F16, tag="prow", bufs=2, name=f"prow{d}")
            for c in range(NCH):
                ps = mmps.tile([128, 512], F32, tag="mm")
                for kt in range(2):
                    nc.tensor.matmul(
                        ps[0:96, :],
                        wt["xpwT"][:, (d * 2 + kt) * 96:(d * 2 + kt + 1) * 96],
                        xbT[kt][:, c * 512:(c + 1) * 512],
                        start=(kt == 0), stop=(kt == 1))
                sl = slice(c * 512, (c + 1) * 512)
                nc.scalar.activation(out=prow[0:8, sl], in_=ps[0:8, :],
                                     func=AF.Tanh)
                nc.scalar.activation(out=prow[32:40, sl], in_=ps[32:40, :],
                                     func=AF.Tanh)
                nc.scalar.activation(out=prow[64:96, sl], in_=ps[64:96, :],
                                     func=AF.Copy)

            # ---- btR/ctR: replicate bt/ct across partitions (s = p%8) ----
            # (btR carries a negated bt; see pat8 prep)
            btR = ap.tile([128, L], F16, tag="btR", bufs=1)
            ctR = ap.tile([128, L], F16, tag="ctR", bufs=1)
            for c in range(NCH):
                sl = slice(c * 512, (c + 1) * 512)
                ps = mmps.tile([128, 512], F32, tag="mm")
                nc.tensor.matmul(ps, wt["pat8"][0:8, :], prow[0:8, sl],
                                 start=True, stop=True)
                nc.scalar.activation(out=btR[:, sl], in_=ps, func=AF.Copy)
                ps2 = mmps.tile([128, 512], F32, tag="mm")
                nc.tensor.matmul(ps2, wt["pat8"][32:40, :], prow[32:40, sl],
                                 start=True, stop=True)
                nc.scalar.activation(out=ctR[:, sl], in_=ps2, func=AF.Copy)

            # ---- delta = softplus(dtwT.T @ dt_raw + dtb) [2 x 128, L] ----
            deltaT = []
            for mb in range(2):
                dt_t = ap.tile([128, L], F16, tag="deltaT", bufs=2)
                deltaT.append(dt_t)
                for c in range(NCH):
                    ps = mmps.tile([128, 512], F32, tag="mm")
                    nc.tensor.matmul(
                        ps, wt["dtwT"][64:96, d * 256 + mb * 128:
                                       d * 256 + (mb + 1) * 128],
                        prow[64:96, c * 512:(c + 1) * 512],
                        start=True, stop=True)
                    # softplus(x + b) = ln(1 + exp(x + b)), exact
                    nc.scalar.activation(
                        out=ps, in_=ps, func=AF.Exp,
                        bias=wt["dtb"][:, 2 * d + mb:2 * d + mb + 1], scale=1.0)
                    nc.scalar.activation(
                        out=dt_t[:, c * 512:(c + 1) * 512], in_=ps,
                        func=AF.Ln, bias=1.0, scale=1.0)
            if d == 0:
                dump("deltaT", deltaT[0])

            # ---- scan core: 16 channel blocks x 2 time halves ----
            yT = [ap.tile([128, L], F16, tag="yT", bufs=2, name=f"yT{d}_{ih}")
                  for ih in range(2)]
            carry = ap.tile([128, 16], F16, tag="carry", bufs=2)
            th_order = (0, 1) if d == 0 else (1, 0)
            ypt = [None, None]
            for thi, th in enumerate(th_order):
                t0 = th * THW
                for k in range(NBLK):
                    ih, j = k // 8, k % 8
                    esl = slice(j * 128, (j + 1) * 128)
                    dx = blkps.tile([128, THW], F32, tag="blk")
                    for h2 in range(2):
                        nc.tensor.matmul(
                            dx[:, h2 * 512:(h2 + 1) * 512], wt["Estk"][:, esl],
                            deltaT[ih][:, t0 + h2 * 512:t0 + (h2 + 1) * 512],
                            start=True, stop=True)
                    d_t = sp.tile([128, THW], F16, tag="d", bufs=2)
                    nc.scalar.activation(
                        out=d_t, in_=dx, func=AF.Exp,
                        scale=wt["negaX"][:, d * 16 + k:d * 16 + k + 1])
                    xx = blkps.tile([128, THW], F32, tag="blk")
                    for h2 in range(2):
                        nc.tensor.matmul(
                            xx[:, h2 * 512:(h2 + 1) * 512], wt["Estk"][:, esl],
                            xbT[ih][:, t0 + h2 * 512:t0 + (h2 + 1) * 512],
                            start=True, stop=True)
                    xbx = sp.tile([128, THW], F16, tag="xbx", bufs=2)
                    nc.scalar.activation(out=xbx, in_=xx, func=AF.Copy)
                    w_t = sp.tile([128, THW], F16, tag="w", bufs=2)
                    nc.vector.tensor_tensor(out=w_t, in0=btR[:, t0:t0 + THW],
                                            in1=xbx, op=OP.mult)
                    # u = (d - 1) * (-bt*xb) = (1-d)*bt*xb, fused on DVE, in place
                    u_t = w_t
                    nc.vector.scalar_tensor_tensor(
                        out=u_t, in0=d_t, scalar=1.0, in1=w_t,
                        op0=OP.subtract, op1=OP.mult)
                    h_t = sp.tile([128, THW], F16, tag="h", bufs=2)
                    init = 0.0 if thi == 0 else carry[:, k:k + 1]
                    if d == 0:
                        nc.vector.tensor_tensor_scan(
                            out=h_t, data0=d_t, data1=u_t, initial=init,
                            op0=OP.mult, op1=OP.add)
                        nc.vector.tensor_copy(carry[:, k:k + 1],
                                              h_t[:, THW - 1:THW])
                    else:
                        nc.vector.tensor_tensor_scan(
                            out=h_t[:, ::-1], data0=d_t[:, ::-1],
                            data1=u_t[:, ::-1], initial=init,
                            op0=OP.mult, op1=OP.add)
                        nc.vector.tensor_copy(carry[:, k:k + 1], h_t[:, 0:1])
                    z_t = h_t
                    nc.vector.tensor_tensor(out=z_t, in0=h_t,
                                            in1=ctR[:, t0:t0 + THW], op=OP.mult)
                    if j == 0:
                        ypt[ih] = yps.tile([128, THW], F32, tag="y",
                                           name=f"yp{d}_{th}_{ih}")
                    for h2 in range(2):
                        nc.tensor.matmul(
                            ypt[ih][:, h2 * 512:(h2 + 1) * 512],
                            wt["Rstk"][:, esl], z_t[:, h2 * 512:(h2 + 1) * 512],
                            start=(j == 0), stop=(j == 7))
                    if j == 7:
                        nc.scalar.activation(out=yT[ih][:, t0:t0 + THW],
                                             in_=ypt[ih], func=AF.Copy)
            if d == 0:
                dump("yT", yT[0])

            # ---- out-proj ----
            rhs_tiles = [yT[0], yT[1], zbT[0], zbT[1], xbT[0], xbT[1]]
            for db in range(4):
                ot = ap.tile([128, L], F16, tag=f"proj{d}_{db}", bufs=1)
                outT.append(ot)
                for c in range(NCH):
                    ps = mmps.tile([128, 512], F32, tag="mm")
                    for kt in range(6):
                        nc.tensor.matmul(
                            ps,
                            wt["owT"][:, (d * 6 + kt) * 512 + db * 128:
                                      (d * 6 + kt) * 512 + (db + 1) * 128],
                            rhs_tiles[kt][:, c * 512:(c + 1) * 512],
                            start=(kt == 0), stop=(kt == 5))
                    nc.scalar.activation(out=ot[:, c * 512:(c + 1) * 512],
                                         in_=ps, func=AF.Copy)
        dump("fwdT", fwdT[0])
        dump("bwdT", bwdT[0])

        # =================================================================
        # Stage F/G: mix matmul (L-layout out) + LN2 + store
        # =================================================================
        mixT_t = wp.tile([128, 12 * 512], F16, tag="mixT")
        nc.sync.dma_start(out=mixT_t, in_=H["mixT"])
        lhs_tiles = fwdT + bwdT + siT
        for m in range(16):
            ps = mmps.tile([128, 512], F32, tag="mm")
            for kt in range(12):
                nc.tensor.matmul(ps, lhs_tiles[kt][:, m * 128:(m + 1) * 128],
                                 mixT_t[:, kt * 512:(kt + 1) * 512],
                                 start=(kt == 0), stop=(kt == 11))
            stats = ap.tile([128, 6], F32, tag="stats2", bufs=3)
            nc.vector.bn_stats(out=stats, in_=ps)
            mv = ap.tile([128, 2], F32, tag="mv2", bufs=3)
            nc.vector.bn_aggr(out=mv, in_=stats)
            nc.scalar.activation(out=mv[:, 1:2], in_=mv[:, 1:2], func=AF.Sqrt,
                                 bias=eps_t, scale=1.0)
            nc.vector.reciprocal(out=mv[:, 1:2], in_=mv[:, 1:2])
            ot = ap.tile([128, D], F32, tag="outL", bufs=2)
            nc.vector.tensor_scalar(out=ot, in0=ps, scalar1=mv[:, 0:1],
                                    scalar2=mv[:, 1:2],
                                    op0=OP.subtract, op1=OP.mult)
            if not ln2_id:
                nc.vector.tensor_tensor(out=ot, in0=ot, in1=wt["ln2w"],
                                        op=OP.mult)
                nc.vector.tensor_tensor(out=ot, in0=ot, in1=wt["ln2b"],
                                        op=OP.add)
            nc.sync.dma_start(out=H["out"][m * 128:(m + 1) * 128, :], in_=ot)


# --------------------------------------------------------------------------
# entry point
# --------------------------------------------------------------------------

def _get_nc(flags):
    key = ("nc", flags, os.environ.get("BASSK_DEBUG", ""))
    if key not in _CACHE:
        _CACHE[key] = _build_nc(flags)
    return _CACHE[key]


def make_in_maps(inputs):
    w, x, position, flags = _prep(inputs)
    shared = {k: v for k, v in w.items() if isinstance(v, np.ndarray)}
    ln1_id, ln2_id, sib_zero = flags
    if ln1_id:
        shared.pop("ln1w"), shared.pop("ln1b")
    if ln2_id:
        shared.pop("ln2w"), shared.pop("ln2b")
    if sib_zero:
        shared.pop("sib")
    in_maps = []
    for b in range(NCORES):
        m = dict(shared)
        m["x"] = np.ascontiguousarray(x[b])
        m["posT"] = np.ascontiguousarray(position[b].T).astype(np.float16)
        in_maps.append(m)
    return in_maps, flags


def kernel(**inputs):
    in_maps, flags = make_in_maps(inputs)
    nc = _get_nc(flags)
    res = run_bass_kernel_spmd(nc, in_maps, list(range(NCORES)))
    out = np.stack([np.asarray(res.results[b]["out"]) for b in range(NCORES)])
    return out.astype(np.float32)


if __name__ == "__main__":
    import time
    t0 = time.time()
    nc = _get_nc((True, True, True))
    print(f"build ok in {time.time() - t0:.1f}s")



# revision 21
# speedup vs baseline: 4565.1410x; 1.0242x over previous
"""Trainium2 Bass kernel for nn_CrossScaleOrthogonalMambaBlock.

Contract: kernel(**inputs) takes the FULL unsharded inputs (numpy), returns the
FULL output [B, L, D] float32.  Internally shards batch-parallel across 8
NeuronCores (1 batch element per core), with all parameters replicated.

Self-contained: hardcodes all shapes; no sibling imports.
"""
import os
import sys
import numpy as np

sys.path.insert(0, "/opt/trn_rl_repo")

import concourse.bass as bass
from concourse import bacc
import concourse.mybir as mybir
import concourse.tile as tile
from concourse.bass_utils import run_bass_kernel_spmd
from concourse.masks import make_identity

# Problem dims
B, L, D, I, S, DTR, KC, PH = 8, 2048, 512, 256, 8, 32, 3, 128
NCORES = 8
NCH = L // 512          # 4 chunks of 512 along time
NBLK = 16               # channel blocks of 128 = (16 i's x 8 s's) per direction
THW = L // 2            # scan processes time in halves of 1024
F16 = mybir.dt.float16
F32 = mybir.dt.float32
AF = mybir.ActivationFunctionType
if os.environ.get("BASSK_SIMFUNCS"):
    class _AFSim:
        def __getattr__(self, n):
            if n == "Gelu": return mybir.ActivationFunctionType.Tanh
            if n == "Silu": return mybir.ActivationFunctionType.Sigmoid
            return getattr(mybir.ActivationFunctionType, n)
    AF = _AFSim()
OP = mybir.AluOpType

_CACHE: dict = {}


# --------------------------------------------------------------------------
# host-side parameter prep (pure layout: transpose / replicate / reorder)
# --------------------------------------------------------------------------

def _softplus(x):
    return np.logaddexp(0.0, x)


def _prep(inputs):
    f = lambda k: np.asarray(inputs[k], np.float32)
    x = f("x")
    position = f("position")
    ln_in_w, ln_in_b = f("ln_in_w"), f("ln_in_b")
    pos_w1, pos_b1 = f("pos_w1"), f("pos_b1")
    pos_w2, pos_b2 = f("pos_w2"), f("pos_b2")
    in_w = f("scan_in_w")
    cx_w, cz_w = f("scan_cx_w"), f("scan_cz_w")
    xp_w = f("scan_xp_w")
    dt_w, dt_b = f("scan_dt_w"), f("scan_dt_b")
    Alog, Dp = f("scan_Alog"), f("scan_Dp")
    out_w = f("scan_out_w")
    mix_w = f("mix_w")
    ln_out_w, ln_out_b = f("ln_out_w"), f("ln_out_b")

    ln1_id = bool(np.allclose(ln_in_w, 1.0) and np.allclose(ln_in_b, 0.0))
    ln2_id = bool(np.allclose(ln_out_w, 1.0) and np.allclose(ln_out_b, 0.0))
    sib_zero = bool(np.allclose(pos_b2, 0.0))

    w = {}
    w["pw1T"] = pos_w1.T.astype(np.float16)                      # [6, 128]
    w["pw2T"] = pos_w2.T.astype(np.float16)                      # [128, 512]
    w["gelub"] = pos_b1.reshape(PH, 1).astype(np.float32)        # [128, 1]
    w["sib"] = pos_b2.reshape(128, 4, order="F").astype(np.float32)  # [128, 4] (col=db)

    # in_w.T as [128, 2*4*512]: slice (d, kt) -> [128, 512]
    inwT = np.zeros((128, 2 * 4 * 512), np.float32)
    for d in range(2):
        t = in_w[d].T                                            # [din, dout]
        for kt in range(4):
            inwT[:, (d * 4 + kt) * 512:(d * 4 + kt + 1) * 512] = \
                t[kt * 128:(kt + 1) * 128, :]
    w["inwT"] = inwT.astype(np.float16)

    # conv taps as diagonal matrices [128, 2*4*3*128]; dir1 taps reversed
    convd = np.zeros((128, 2 * 4 * KC * 128), np.float32)
    for d in range(2):
        cw = np.concatenate([cx_w[d, :, 0, :], cz_w[d, :, 0, :]], 0)  # [512, 3]
        if d == 1:
            cw = cw[:, ::-1]
        for g in range(4):
            for t in range(KC):
                off = ((d * 4 + g) * KC + t) * 128
                dg = np.zeros((128, 128), np.float32)
                np.fill_diagonal(dg, cw[g * 128:(g + 1) * 128, t])
                convd[:, off:off + 128] = dg
    w["convd"] = convd.astype(np.float16)

    # xproj, reordered cols: 0:8 = b_raw, 32:40 = c_raw, 64:96 = dt_raw
    # layout [128, 2*2*96]: slice (d, kt) -> [128, 96]
    xpe = np.zeros((2, I, 96), np.float32)
    xpe[:, :, 0:8] = np.transpose(xp_w[:, DTR:DTR + S, :], (0, 2, 1))
    xpe[:, :, 32:40] = np.transpose(xp_w[:, DTR + S:DTR + 2 * S, :], (0, 2, 1))
    xpe[:, :, 64:96] = np.transpose(xp_w[:, 0:DTR, :], (0, 2, 1))
    xpwT = np.zeros((128, 2 * 2 * 96), np.float32)
    for d in range(2):
        for kt in range(2):
            xpwT[:, (d * 2 + kt) * 96:(d * 2 + kt + 1) * 96] = \
                xpe[d, kt * 128:(kt + 1) * 128, :]
    w["xpwT"] = xpwT.astype(np.float16)

    # dt_w.T [32, 2*256]: slice (d, mb) -> [32, 128]
    dtwT = np.zeros((96, 2 * 256), np.float32)
    for d in range(2):
        dtwT[64:96, d * 256:(d + 1) * 256] = dt_w[d].T
    w["dtwT"] = dtwT.astype(np.float16)
    # dt_b [128, 4]: col = d*2 + mb
    w["dtb"] = np.ascontiguousarray(
        dt_b.reshape(2, 2, 128).transpose(2, 0, 1).reshape(128, 4)).astype(np.float32)

    # expansion / reduction / replication masks
    p = np.arange(128)
    Estk = np.zeros((128, 8, 128), np.float32)   # E_j[r, p] = r == 16j + p//8
    Rstk = np.zeros((128, 8, 128), np.float32)   # R_j[p, m] = m == 16j + p//8
    for j in range(8):
        Estk[16 * j + p // 8, j, p] = 1.0
        Rstk[p, j, 16 * j + p // 8] = 1.0
    w["Estk"] = Estk.reshape(128, 8 * 128).astype(np.float16)
    w["Rstk"] = Rstk.reshape(128, 8 * 128).astype(np.float16)
    pat8 = np.zeros((128, 128), np.float32)
    # bt replication negated: btR = -bt, so u = (d-1)*(btR*xb) = (1-d)*bt*xb
    pat8[0:8, :] = -(np.arange(8)[:, None] == (p % 8)[None, :]).astype(np.float32)
    pat8[32:40, :] = (np.arange(8)[:, None] == (p % 8)[None, :])
    w["pat8"] = pat8.astype(np.float16)

    a = _softplus(Alog) + 1e-4                                   # [2, 256, 8]
    negaX = np.zeros((128, 32), np.float32)                      # col = d*16 + k
    for d in range(2):
        for k in range(16):
            negaX[:, d * 16 + k] = -a[d, 16 * k + p // 8, p % 8]
    w["negaX"] = negaX

    # out-proj extended rows [y(0:256); zb(256:512); xb*Dp(512:768)]
    # layout [128, 2*6*512]: slice (d, kt) -> [128, 512]
    owT = np.zeros((128, 2 * 6 * 512), np.float32)
    for d in range(2):
        ext = np.concatenate([out_w[d].T, out_w[d].T[0:256] * Dp[d][:, None]], 0)
        for kt in range(6):
            owT[:, (d * 6 + kt) * 512:(d * 6 + kt + 1) * 512] = \
                ext[kt * 128:(kt + 1) * 128, :]
    w["owT"] = owT.astype(np.float16)

    # mix_w.T [128, 12*512]: slice kt -> [128, 512]
    mixT = np.zeros((128, 12 * 512), np.float32)
    mt = mix_w.T                                                 # [1536, 512]
    for kt in range(12):
        mixT[:, kt * 512:(kt + 1) * 512] = mt[kt * 128:(kt + 1) * 128, :]
    w["mixT"] = mixT.astype(np.float16)

    w["ln1w"] = np.broadcast_to(ln_in_w, (128, D)).astype(np.float32).copy()
    w["ln1b"] = np.broadcast_to(ln_in_b, (128, D)).astype(np.float32).copy()
    w["ln2w"] = np.broadcast_to(ln_out_w, (128, D)).astype(np.float32).copy()
    w["ln2b"] = np.broadcast_to(ln_out_b, (128, D)).astype(np.float32).copy()

    flags = (ln1_id, ln2_id, sib_zero)
    return w, x, position, flags


# --------------------------------------------------------------------------
# device program
# --------------------------------------------------------------------------

WEIGHT_SHAPES = {
    "pw1T": ([6, PH], F16), "pw2T": ([PH, D], F16), "gelub": ([PH, 1], F32),
    "sib": ([128, 4], F32), "inwT": ([128, 2 * 4 * 512], F16),
    "convd": ([128, 2 * 4 * KC * 128], F16), "xpwT": ([128, 2 * 2 * 96], F16),
    "dtwT": ([96, 2 * 256], F16), "dtb": ([128, 4], F32),
    "Estk": ([128, 8 * 128], F16), "Rstk": ([128, 8 * 128], F16),
    "pat8": ([128, 128], F16), "negaX": ([128, 32], F32),
    "owT": ([128, 2 * 6 * 512], F16), "mixT": ([128, 12 * 512], F16),
    "ln1w": ([128, D], F32), "ln1b": ([128, D], F32),
    "ln2w": ([128, D], F32), "ln2b": ([128, D], F32),
}


def _build_nc(flags):
    ln1_id, ln2_id, sib_zero = flags
    nc = bacc.Bacc()

    def par(name, shape, dtype=F16, out=False):
        return nc.declare_dram_parameter(name, list(shape), dtype, isOutput=out).ap()

    H = {"flags": flags}
    H["x"] = par("x", [L, D], F32)
    H["posT"] = par("posT", [6, L], F16)
    skip = set()
    if ln1_id:
        skip |= {"ln1w", "ln1b"}
    if ln2_id:
        skip |= {"ln2w", "ln2b"}
    if sib_zero:
        skip |= {"sib"}
    H["skip"] = skip
    for nm, (shape, dt) in WEIGHT_SHAPES.items():
        if nm in skip:
            continue
        H[nm] = par(nm, shape, dt)
    H["out"] = par("out", [L, D], F32, out=True)

    dbg = os.environ.get("BASSK_DEBUG", "")
    H["dbg_outs"] = {}
    for spec in [s for s in dbg.split(",") if s]:
        name, *shape = spec.split(":")
        H["dbg_outs"][name] = par("dbg_" + name, [int(s) for s in shape],
                                  F16, out=True)

    with tile.TileContext(nc) as tc:
        _body(tc, H)
    nc.compile()
    return nc


def _body(tc, H):
    nc = tc.nc
    ln1_id, ln2_id, sib_zero = H["flags"]
    dbg_outs = H["dbg_outs"]

    def dump(name, src):
        if name in dbg_outs:
            nc.sync.dma_start(out=dbg_outs[name], in_=src)

    with (
        tc.tile_pool(name="wp", bufs=1) as wp,
        tc.tile_pool(name="ap", bufs=2) as ap,
        tc.tile_pool(name="sp", bufs=2) as sp,
        tc.tile_pool(name="mmps", bufs=2, space="PSUM") as mmps,
        tc.tile_pool(name="blkps", bufs=2, space="PSUM") as blkps,
        tc.tile_pool(name="yps", bufs=1, space="PSUM") as yps,
    ):
        # ---- persistent weights in SBUF ----
        wt = {}
        for nm, (shape, dt) in WEIGHT_SHAPES.items():
            if nm in H["skip"] or nm == "mixT":
                continue
            t = wp.tile(shape, dt, tag=nm)
            nc.sync.dma_start(out=t, in_=H[nm])
            wt[nm] = t

        ident32 = wp.tile([128, 128], F32, tag="ident32")
        make_identity(nc, ident32)
        eps_t = wp.tile([128, 1], F32, tag="eps")
        nc.vector.memset(eps_t, 1e-6)
        posT_t = wp.tile([6, L], F16, tag="posT")
        nc.sync.dma_start(out=posT_t, in_=H["posT"])

        # =================================================================
        # Stage A/B: LN1 of x (L-layout) -> normL fp16 [128, 512] x 16
        # =================================================================
        normL = []
        for m in range(16):
            xt = ap.tile([128, D], F32, tag="xin", bufs=2)
            nc.sync.dma_start(out=xt, in_=H["x"][m * 128:(m + 1) * 128, :])
            stats = ap.tile([128, 6], F32, tag="stats", bufs=3)
            nc.vector.bn_stats(out=stats, in_=xt)
            mv = ap.tile([128, 2], F32, tag="mv", bufs=3)
            nc.vector.bn_aggr(out=mv, in_=stats)
            nc.scalar.activation(out=mv[:, 1:2], in_=mv[:, 1:2], func=AF.Sqrt,
                                 bias=eps_t, scale=1.0)
            nc.vector.reciprocal(out=mv[:, 1:2], in_=mv[:, 1:2])
            nt = ap.tile([128, D], F32, tag="normL", bufs=6)
            if ln1_id:
                nc.vector.tensor_scalar(out=nt, in0=xt, scalar1=mv[:, 0:1],
                                        scalar2=mv[:, 1:2],
                                        op0=OP.subtract, op1=OP.mult)
            else:
                tmp = ap.tile([128, D], F32, tag="lntmp", bufs=2)
                nc.vector.tensor_scalar(out=tmp, in0=xt, scalar1=mv[:, 0:1],
                                        scalar2=mv[:, 1:2],
                                        op0=OP.subtract, op1=OP.mult)
                nc.vector.tensor_tensor(out=tmp, in0=tmp, in1=wt["ln1w"],
                                        op=OP.mult)
                nc.vector.tensor_tensor(out=nt, in0=tmp, in1=wt["ln1b"],
                                        op=OP.add)
            normL.append(nt)

        # =================================================================
        # Stage C: pos MLP layer 1: h1T = gelu(pw1T.T @ posT + b1)
        # =================================================================
        h1T = ap.tile([128, L], F16, tag="prow", bufs=2)
        for c in range(NCH):
            ps = mmps.tile([128, 512], F32, tag="mm")
            nc.tensor.matmul(ps, wt["pw1T"], posT_t[:, c * 512:(c + 1) * 512],
                             start=True, stop=True)
            nc.scalar.activation(out=h1T[:, c * 512:(c + 1) * 512], in_=ps,
                                 func=AF.Gelu, bias=wt["gelub"], scale=1.0)

        # =================================================================
        # Stage D: siT[db] = blockwise-transpose(normL) + pw2T.T @ h1T (+sib)
        # =================================================================
        siT = [ap.tile([128, L], F16, tag=f"siT{db}", bufs=1, name=f"siT{db}")
               for db in range(4)]
        for c in range(NCH):
            for db in range(4):
                st = siT[db]
                ps = mmps.tile([128, 512], F32, tag="mm")
                nc.tensor.matmul(ps, wt["pw2T"][:, db * 128:(db + 1) * 128],
                                 h1T[:, c * 512:(c + 1) * 512],
                                 start=True, stop=False)
                for q in range(4):
                    nc.tensor.matmul(ps[:, q * 128:(q + 1) * 128],
                                     normL[c * 4 + q][:, db * 128:(db + 1) * 128],
                                     ident32, is_transpose=True,
                                     start=False, stop=(q == 3))
                dst = st[:, c * 512:(c + 1) * 512]
                if sib_zero:
                    nc.scalar.activation(out=dst, in_=ps, func=AF.Copy)
                else:
                    nc.scalar.activation(out=dst, in_=ps, func=AF.Identity,
                                         bias=wt["sib"][:, db:db + 1], scale=1.0)
        dump("siT", siT[0])

        # =================================================================
        # Stage E: per-direction scan pipeline
        # =================================================================
        fwdT, bwdT = [], []
        for d in range(2):
            outT = fwdT if d == 0 else bwdT

            # ---- in-proj into padded conv-input tiles ----
            xzpad = [None] * 4
            xbT, zbT = [], []

            def inproj(gs):
                for g in gs:
                    xt = ap.tile([128, L + 2], F16, tag="xzpad", bufs=4)
                    xzpad[g] = xt
                    nc.gpsimd.memset(xt[:, 0:1], 0.0)
                    nc.gpsimd.memset(xt[:, L + 1:L + 2], 0.0)
                    for c in range(NCH):
                        ps = mmps.tile([128, 512], F32, tag="mm")
                        for kt in range(4):
                            nc.tensor.matmul(
                                ps,
                                wt["inwT"][:, (d * 4 + kt) * 512 + g * 128:
                                           (d * 4 + kt) * 512 + (g + 1) * 128],
                                siT[kt][:, c * 512:(c + 1) * 512],
                                start=(kt == 0), stop=(kt == 3))
                        nc.scalar.activation(
                            out=xt[:, 1 + c * 512:1 + (c + 1) * 512],
                            in_=ps, func=AF.Copy)

            def convs(gs):
                # depthwise conv (diag matmuls) + SiLU
                for g in gs:
                    ot = ap.tile([128, L], F16, tag="xbzb", bufs=5)
                    (xbT if g < 2 else zbT).append(ot)
                    for c in range(NCH):
                        ps = mmps.tile([128, 512], F32, tag="mm")
                        for t in range(KC):
                            off = ((d * 4 + g) * KC + t) * 128
                            nc.tensor.matmul(
                                ps, wt["convd"][:, off:off + 128],
                                xzpad[g][:, c * 512 + t:c * 512 + t + 512],
                                start=(t == 0), stop=(t == KC - 1))
                        nc.scalar.activation(out=ot[:, c * 512:(c + 1) * 512],
                                             in_=ps, func=AF.Silu)

            # xb half first (feeds xproj/delta/scan); zb half deferred
            inproj([0, 1])
            convs([0, 1])
            if d == 0:
                dump("xbT", xbT[0])

            # ---- xproj -> prow: bt(0:8), ct(32:40), dt_raw(64:96) ----
            prow = ap.tile([128, L], F16, tag="prow", bufs=2, name=f"prow{d}")
            for c in range(NCH):
                ps = mmps.tile([128, 512], F32, tag="mm")
                for kt in range(2):
                    nc.tensor.matmul(
                        ps[0:96, :],
                        wt["xpwT"][:, (d * 2 + kt) * 96:(d * 2 + kt + 1) * 96],
                        xbT[kt][:, c * 512:(c + 1) * 512],
                        start=(kt == 0), stop=(kt == 1))
                sl = slice(c * 512, (c + 1) * 512)
                nc.scalar.activation(out=prow[0:8, sl], in_=ps[0:8, :],
                                     func=AF.Tanh)
                nc.scalar.activation(out=prow[32:40, sl], in_=ps[32:40, :],
                                     func=AF.Tanh)
                nc.scalar.activation(out=prow[64:96, sl], in_=ps[64:96, :],
                                     func=AF.Copy)

            # ---- btR/ctR: replicate bt/ct across partitions (s = p%8) ----
            # (btR carries a negated bt; see pat8 prep)
            btR = ap.tile([128, L], F16, tag="btR", bufs=1)
            ctR = ap.tile([128, L], F16, tag="ctR", bufs=1)
            for c in range(NCH):
                sl = slice(c * 512, (c + 1) * 512)
                ps = mmps.tile([128, 512], F32, tag="mm")
                nc.tensor.matmul(ps, wt["pat8"][0:8, :], prow[0:8, sl],
                                 start=True, stop=True)
                nc.scalar.activation(out=btR[:, sl], in_=ps, func=AF.Copy)
                ps2 = mmps.tile([128, 512], F32, tag="mm")
                nc.tensor.matmul(ps2, wt["pat8"][32:40, :], prow[32:40, sl],
                                 start=True, stop=True)
                nc.scalar.activation(out=ctR[:, sl], in_=ps2, func=AF.Copy)

            # ---- delta = softplus(dtwT.T @ dt_raw + dtb) [2 x 128, L] ----
            deltaT = []
            for mb in range(2):
                dt_t = ap.tile([128, L], F16, tag="deltaT", bufs=2)
                deltaT.append(dt_t)
                for c in range(NCH):
                    ps = mmps.tile([128, 512], F32, tag="mm")
                    nc.tensor.matmul(
                        ps, wt["dtwT"][64:96, d * 256 + mb * 128:
                                       d * 256 + (mb + 1) * 128],
                        prow[64:96, c * 512:(c + 1) * 512],
                        start=True, stop=True)
                    # softplus(x + b) = ln(1 + exp(x + b)), exact
                    nc.scalar.activation(
                        out=ps, in_=ps, func=AF.Exp,
                        bias=wt["dtb"][:, 2 * d + mb:2 * d + mb + 1], scale=1.0)
                    nc.scalar.activation(
                        out=dt_t[:, c * 512:(c + 1) * 512], in_=ps,
                        func=AF.Ln, bias=1.0, scale=1.0)
            if d == 0:
                dump("deltaT", deltaT[0])

            # zb half: not needed until out-proj; fills PE idle during scans
            inproj([2, 3])
            convs([2, 3])

            # ---- scan core: 16 channel blocks x 2 time halves ----
            yT = [ap.tile([128, L], F16, tag="yT", bufs=2, name=f"yT{d}_{ih}")
                  for ih in range(2)]
            carry = ap.tile([128, 16], F16, tag="carry", bufs=2)
            th_order = (0, 1) if d == 0 else (1, 0)
            ypt = [None, None]
            for thi, th in enumerate(th_order):
                t0 = th * THW
                for k in range(NBLK):
                    ih, j = k // 8, k % 8
                    esl = slice(j * 128, (j + 1) * 128)
                    dx = blkps.tile([128, THW], F32, tag="blk")
                    for h2 in range(2):
                        nc.tensor.matmul(
                            dx[:, h2 * 512:(h2 + 1) * 512], wt["Estk"][:, esl],
                            deltaT[ih][:, t0 + h2 * 512:t0 + (h2 + 1) * 512],
                            start=True, stop=True)
                    d_t = sp.tile([128, THW], F16, tag="d", bufs=2)
                    nc.scalar.activation(
                        out=d_t, in_=dx, func=AF.Exp,
                        scale=wt["negaX"][:, d * 16 + k:d * 16 + k + 1])
                    xx = blkps.tile([128, THW], F32, tag="blk")
                    for h2 in range(2):
                        nc.tensor.matmul(
                            xx[:, h2 * 512:(h2 + 1) * 512], wt["Estk"][:, esl],
                            xbT[ih][:, t0 + h2 * 512:t0 + (h2 + 1) * 512],
                            start=True, stop=True)
                    xbx = sp.tile([128, THW], F16, tag="xbx", bufs=2)
                    nc.scalar.activation(out=xbx, in_=xx, func=AF.Copy)
                    w_t = sp.tile([128, THW], F16, tag="w", bufs=2)
                    nc.vector.tensor_tensor(out=w_t, in0=btR[:, t0:t0 + THW],
                                            in1=xbx, op=OP.mult)
                    # u = (d - 1) * (-bt*xb) = (1-d)*bt*xb, fused on DVE, in place
                    u_t = w_t
                    nc.vector.scalar_tensor_tensor(
                        out=u_t, in0=d_t, scalar=1.0, in1=w_t,
                        op0=OP.subtract, op1=OP.mult)
                    h_t = sp.tile([128, THW], F16, tag="h", bufs=2)
                    init = 0.0 if thi == 0 else carry[:, k:k + 1]
                    if d == 0:
                        nc.vector.tensor_tensor_scan(
                            out=h_t, data0=d_t, data1=u_t, initial=init,
                            op0=OP.mult, op1=OP.add)
                        if thi == 0:
                            nc.scalar.activation(out=carry[:, k:k + 1],
                                                 in_=h_t[:, THW - 1:THW],
                                                 func=AF.Copy)
                    else:
                        nc.vector.tensor_tensor_scan(
                            out=h_t[:, ::-1], data0=d_t[:, ::-1],
                            data1=u_t[:, ::-1], initial=init,
                            op0=OP.mult, op1=OP.add)
                        if thi == 0:
                            nc.scalar.activation(out=carry[:, k:k + 1],
                                                 in_=h_t[:, 0:1],
                                                 func=AF.Copy)
                    z_t = h_t
                    nc.vector.tensor_tensor(out=z_t, in0=h_t,
                                            in1=ctR[:, t0:t0 + THW], op=OP.mult)
                    if j == 0:
                        ypt[ih] = yps.tile([128, THW], F32, tag="y",
                                           name=f"yp{d}_{th}_{ih}")
                    for h2 in range(2):
                        nc.tensor.matmul(
                            ypt[ih][:, h2 * 512:(h2 + 1) * 512],
                            wt["Rstk"][:, esl], z_t[:, h2 * 512:(h2 + 1) * 512],
                            start=(j == 0), stop=(j == 7))
                    if j == 7:
                        nc.scalar.activation(out=yT[ih][:, t0:t0 + THW],
                                             in_=ypt[ih], func=AF.Copy)
            if d == 0:
                dump("yT", yT[0])

            # ---- out-proj ----
            rhs_tiles = [yT[0], yT[1], zbT[0], zbT[1], xbT[0], xbT[1]]
            for db in range(4):
                ot = ap.tile([128, L], F16, tag=f"proj{d}_{db}", bufs=1)
                outT.append(ot)
                for c in range(NCH):
                    ps = mmps.tile([128, 512], F32, tag="mm")
                    for kt in range(6):
                        nc.tensor.matmul(
                            ps,
                            wt["owT"][:, (d * 6 + kt) * 512 + db * 128:
                                      (d * 6 + kt) * 512 + (db + 1) * 128],
                            rhs_tiles[kt][:, c * 512:(c + 1) * 512],
                            start=(kt == 0), stop=(kt == 5))
                    nc.scalar.activation(out=ot[:, c * 512:(c + 1) * 512],
                                         in_=ps, func=AF.Copy)
        dump("fwdT", fwdT[0])
        dump("bwdT", bwdT[0])

        # =================================================================
        # Stage F/G: mix matmul (L-layout out) + LN2 + store
        # =================================================================
        mixT_t = wp.tile([128, 12 * 512], F16, tag="mixT")
        nc.sync.dma_start(out=mixT_t, in_=H["mixT"])
        lhs_tiles = fwdT + bwdT + siT
        for m in range(16):
            ps = mmps.tile([128, 512], F32, tag="mm")
            for kt in range(12):
                nc.tensor.matmul(ps, lhs_tiles[kt][:, m * 128:(m + 1) * 128],
                                 mixT_t[:, kt * 512:(kt + 1) * 512],
                                 start=(kt == 0), stop=(kt == 11))
            stats = ap.tile([128, 6], F32, tag="stats2", bufs=3)
            nc.vector.bn_stats(out=stats, in_=ps)
            mv = ap.tile([128, 2], F32, tag="mv2", bufs=3)
            nc.vector.bn_aggr(out=mv, in_=stats)
            nc.scalar.activation(out=mv[:, 1:2], in_=mv[:, 1:2], func=AF.Sqrt,
                                 bias=eps_t, scale=1.0)
            nc.vector.reciprocal(out=mv[:, 1:2], in_=mv[:, 1:2])
            ot = ap.tile([128, D], F32, tag="outL", bufs=2)
            nc.vector.tensor_scalar(out=ot, in0=ps, scalar1=mv[:, 0:1],
                                    scalar2=mv[:, 1:2],
                                    op0=OP.subtract, op1=OP.mult)
            if not ln2_id:
                nc.vector.tensor_tensor(out=ot, in0=ot, in1=wt["ln2w"],
                                        op=OP.mult)
                nc.vector.tensor_tensor(out=ot, in0=ot, in1=wt["ln2b"],
                                        op=OP.add)
            nc.sync.dma_start(out=H["out"][m * 128:(m + 1) * 128, :], in_=ot)


# --------------------------------------------------------------------------
# entry point
# --------------------------------------------------------------------------

def _get_nc(flags):
    key = ("nc", flags, os.environ.get("BASSK_DEBUG", ""))
    if key not in _CACHE:
        _CACHE[key] = _build_nc(flags)
    return _CACHE[key]


def make_in_maps(inputs):
    w, x, position, flags = _prep(inputs)
    shared = {k: v for k, v in w.items() if isinstance(v, np.ndarray)}
    ln1_id, ln2_id, sib_zero = flags
    if ln1_id:
        shared.pop("ln1w"), shared.pop("ln1b")
    if ln2_id:
        shared.pop("ln2w"), shared.pop("ln2b")
    if sib_zero:
        shared.pop("sib")
    in_maps = []
    for b in range(NCORES):
        m = dict(shared)
        m["x"] = np.ascontiguousarray(x[b])
        m["posT"] = np.ascontiguousarray(position[b].T).astype(np.float16)
        in_maps.append(m)
    return in_maps, flags


def kernel(**inputs):
    in_maps, flags = make_in_maps(inputs)
    nc = _get_nc(flags)
    res = run_bass_kernel_spmd(nc, in_maps, list(range(NCORES)))
    out = np.stack([np.asarray(res.results[b]["out"]) for b in range(NCORES)])
    return out.astype(np.float32)


if __name__ == "__main__":
    import time
    t0 = time.time()
    nc = _get_nc((True, True, True))
    print(f"build ok in {time.time() - t0:.1f}s")

